# revision 36
# baseline (speedup 1.0000x reference)
"""Trainium2 Bass kernel for nn_Attention_37641093382387.

Dense transformer attention block:
  qkv = x @ Wqkv; q,k + RoPE; causal softmax attention; out @ Wproj + bproj

Sharding: 8 cores = 2 batches x 4 head-groups (4 heads each).  Each core
computes its batch's partial output for its head group; host sums the 4
group partials per batch and adds the bias.

Per-core device pipeline (all matmuls bf16 -> f32 PSUM):
  - host passes x^T (pre-transposed, bf16) so no on-chip transposes needed
  - qT,kT computed in [hd, T] layout (lhsT=W block, rhs=xT);
    v in [T, hd] layout (lhsT=xT block, rhs=Wv)
  - RoPE rotate-half done via a permutation matmul on PE (DVE has no
    cross-partition path; signs folded into the sin table), then 3 DVE ops
  - attention computed transposed: ST[s,t] = kT_tile^T @ qT -> exp on ACT
    (scale folded into exp) -> PT bf16; row sums via all-ones matmul
    (replicated across partitions); OT accum = v_tile^T @ PT;
    normalization via DVE reciprocal+mul; causal handled by narrowing
    matmuls to the valid t-range + one 128x128 triangle mask multiply
  - row sums for t>=512 blocks: fp8e4 DoubleRow matmuls over s-tile pairs
    (2x PE rate; DVE makes a 0.25-scaled fp8 copy of PT, the ones tensor
    carries 4.0 to undo it).  j=0 rows have few softmax terms where fp8
    relative sum error is largest, so they stay bf16.  The whole pss bank
    is ONE accumulation group armed by a single start=True.
  - proj: Y = OT^T blocks @ Wproj, f32 out
  - cos/sin and rot/tri/ones are packed into single dram tensors (each
    dma_start costs ~600ns of serialized SP issue time at kernel start)
"""

import os
import sys
from collections import deque

import numpy as np

for _p in ("/opt/trn_rl_repo",):
    if _p not in sys.path and os.path.isdir(_p):
        sys.path.insert(0, _p)

import ml_dtypes

bf16 = ml_dtypes.bfloat16
f8e4 = ml_dtypes.float8_e4m3

P = 128
T = 2048
D = 2048
HD = 128
NG = 4      # head groups
HPG = 4     # heads per group
B = 2
BK = 512    # t block
NB = T // BK          # 4 t-blocks
NKT = D // P          # 16 contraction chunks
NTT = T // P          # 16 t-tiles
SCALE = float(HD) ** -0.5
E4SCALE = 0.25        # pt -> fp8 scale (max exp ~693 -> 173 < 240)
ONES8VAL = 4.0        # undoes E4SCALE in the row-sum matmul

_NC_CACHE = {}


def _build_nc():
    import concourse.mybir as mybir
    from concourse import bacc
    from concourse.tile import TileContext

    fp32 = mybir.dt.float32
    bf = mybir.dt.bfloat16
    e4 = mybir.dt.float8e4
    Exp = mybir.ActivationFunctionType.Exp
    DR = mybir.MatmulPerfMode.DoubleRow

    nc = bacc.Bacc("TRN2", target_bir_lowering=False, debug=False,
                   num_devices=B * NG)

    xt_d = nc.declare_dram_parameter("xt", [NB, P, NKT, BK], bf,
                                     isOutput=False)
    wqk_d = nc.declare_dram_parameter("wqk", [2 * HPG, P, NKT, HD], bf,
                                      isOutput=False)
    wv_d = nc.declare_dram_parameter("wv", [P, NKT, HPG * HD], bf,
                                     isOutput=False)
    wp_d = nc.declare_dram_parameter("wp", [P, HPG, D], bf, isOutput=False)
    # cos/sin packed into one tensor, rot/tri/ones into another: each extra
    # dma_start costs ~600ns of serialized SP issue time at kernel start
    cs_d = nc.declare_dram_parameter("cs", [HD, 2, T], bf, isOutput=False)
    cst_d = nc.declare_dram_parameter("cst", [P, 3, P], bf, isOutput=False)
    ones8_d = nc.declare_dram_parameter("ones8", [P, 2, P], e4, isOutput=False)
    out_d = nc.declare_dram_parameter("out", [T, D], bf, isOutput=True)


    with TileContext(nc) as tc, \
         tc.tile_pool(name="const", bufs=1) as constp, \
         tc.tile_pool(name="persist", bufs=1) as persistp, \
         tc.tile_pool(name="xt", bufs=2) as xtp, \
         tc.tile_pool(name="qblk", bufs=2) as qp, \
         tc.tile_pool(name="otblk", bufs=2) as otp, \
         tc.tile_pool(name="work", bufs=3) as workp, \
         tc.tile_pool(name="yout", bufs=6) as ypool, \
         tc.tile_pool(name="pt", bufs=5) as ptp, \
         tc.tile_pool(name="pt8", bufs=3) as pt8p, \
         tc.tile_pool(name="psmm", bufs=3, space="PSUM") as psmm, \
         tc.tile_pool(name="psrot", bufs=1, space="PSUM") as psrot, \
         tc.tile_pool(name="pssum", bufs=2, space="PSUM") as pssum, \
         tc.tile_pool(name="pso", bufs=2, space="PSUM") as psop:

        # ---- PE warmup ----
        # HAM clock-gates the PE to 1.2 GHz until ~3.4us of sustained matmul
        # activity; the first real MMs are DMA-paced until ~10us, so run a
        # dummy chain during the DMA head to enter the warm state for free.
        wu_sb = constp.tile([P, BK], bf, name="warmup")
        nc.gpsimd.memset(wu_sb[:], 0.0)
        wups = psmm.tile([P, BK], fp32, tag="mm", name="wups")
        NWU = 20
        for i in range(NWU):
            nc.tensor.matmul(wups[:], wu_sb[:, 0:P], wu_sb[:],
                             start=(i == 0), stop=(i == NWU - 1))

        # ---- constants ----
        # Chunked loads so the first QKV matmuls only wait on small pieces,
        # and independent chunks spread across DMA queues.

        xt_sb0 = xtp.tile([P, NKT, BK], bf, tag="xt", name="xt_sb0")
        wqk_sb = constp.tile([P, 2 * HPG, NKT, HD], bf)
        cs_sb = constp.tile([HD, 2, T], bf)
        cst_sb = constp.tile([P, 3, P], bf)
        wv_sb = constp.tile([P, NKT, HPG * HD], bf)
        wp_sb = constp.tile([P, HPG, D], bf)
        ones8_sb = constp.tile([P, 2, P], e4)
        # SP issues one dma_start per ~600ns, serialized: keep the count low
        # and emit in j0 consumption order (e-chains 0,4,1,5 | v | attn
        # h0,h1 | 2,6,3,7).  First chunks are small so e_chain(0) can start
        # as early as possible.
        # e0/e4 run kt-interleaved (pair-paced), so their weight chunks must
        # alternate in the stream to arrive before the xt chunk they gate.
        nc.sync.dma_start(wqk_sb[:, 0, 0:4, :], wqk_d[0, :, 0:4, :])
        nc.sync.dma_start(xt_sb0[:, 0:4, :], xt_d[0, :, 0:4, :])
        nc.sync.dma_start(wqk_sb[:, 4, 0:4, :], wqk_d[4, :, 0:4, :])
        nc.sync.dma_start(xt_sb0[:, 4:8, :], xt_d[0, :, 4:8, :])
        nc.sync.dma_start(wqk_sb[:, 0, 4:8, :], wqk_d[0, :, 4:8, :])
        nc.sync.dma_start(wqk_sb[:, 4, 4:8, :], wqk_d[4, :, 4:8, :])
        nc.sync.dma_start(xt_sb0[:, 8:12, :], xt_d[0, :, 8:12, :])
        nc.sync.dma_start(wqk_sb[:, 0, 8:16, :], wqk_d[0, :, 8:16, :])
        nc.sync.dma_start(wqk_sb[:, 4, 8:16, :], wqk_d[4, :, 8:16, :])
        nc.sync.dma_start(xt_sb0[:, 12:16, :], xt_d[0, :, 12:16, :])
        nc.sync.dma_start(cs_sb[:, :, 0:BK], cs_d[:, :, 0:BK])
        nc.sync.dma_start(wqk_sb[:, 1, :, :], wqk_d[1, :, :, :])
        nc.sync.dma_start(wqk_sb[:, 5, :, :], wqk_d[5, :, :, :])
        # wv in chunks so v_chain(0) starts after the first kt slice lands
        for c in range(4):
            nc.sync.dma_start(wv_sb[:, 4 * c:4 * (c + 1), :],
                              wv_d[:, 4 * c:4 * (c + 1), :])
        nc.sync.dma_start(cst_sb[:], cst_d[:])
        nc.sync.dma_start(ones8_sb[:], ones8_d[:])
        for e in (2, 6):
            nc.sync.dma_start(wqk_sb[:, e, :, :], wqk_d[e, :, :, :])
        # xt block 1 prefetch: needed by block 0's interleaved v-chain
        # fillers (~t=60us); issued mid-weight-stream so e3/e7 weights are
        # only delayed by its 2.1MB transfer, still well before their use.
        xt_sb1 = xtp.tile([P, NKT, BK], bf, tag="xt", name="xt_sb1")
        nc.sync.dma_start(xt_sb1[:], xt_d[1, :, :, :])
        for e in (3, 7):
            nc.sync.dma_start(wqk_sb[:, e, :, :], wqk_d[e, :, :, :])
        nc.sync.dma_start(cs_sb[:, :, BK:], cs_d[:, :, BK:])
        nc.sync.dma_start(wp_sb[:], wp_d[:])

        # ---- persistent tensors ----
        k_sb = persistp.tile([HD, HPG, T], bf)        # kT per head
        v_sb = persistp.tile([P, NTT, HPG * HD], bf)  # v  per t-tile
        xt_next = xt_sb1

        for j in range(NB):
            tsl = slice(j * BK, (j + 1) * BK)

            if j == 0:
                xt_sb = xt_sb0
            else:
                xt_sb = xt_next

            q_sb = qp.tile([HD, HPG, BK], bf, tag="qblk")
            ot_sb = otp.tile([HD, HPG, BK], bf, tag="otblk")
            ni = 4 * j + 4
            state = {"pending": []}

            # RoPE for e-tile `e` is emitted after the next matmul chain so
            # the PE never stalls on the ACT psum->bf16 copy.
            def rope_tail(e, t1, raw):
                # the deferred (rot-dependent) half of RoPE; t1 = ps*cos ran
                # at chain end.  The rot matmul keeps NORMAL priority: at
                # priority 0 it would preempt the next attn head's STs in
                # the static PE order and stall the PE on its raw-copy
                # input.  Its DVE consumers are high priority so q/k land
                # promptly once the rot drains.
                psr = psrot.tile([P, BK], fp32, tag="rot", name="psr")
                nc.tensor.matmul(psr[:], cst_sb[:, 0, :], raw[:],
                                 start=True, stop=True)
                with tc.high_priority():
                    t2 = workp.tile([P, BK], fp32, tag="t2", name="t2")
                    nc.vector.tensor_mul(t2[:], psr[:], cs_sb[:, 1, tsl])
                    if e < HPG:
                        dst = q_sb[:, e, :]
                    else:
                        dst = k_sb[:, e - HPG, tsl]
                    nc.vector.tensor_add(dst, t1[:], t2[:])

            def flush_rope():
                # Flush ALL pending ropes.  Call sites are chosen away from
                # attn-head boundaries: a rope's rot matmul churns the psrot
                # bank, which the next head's first ST borrows.  Back-to-back
                # rots serialize on psrot but the scheduler absorbs that with
                # other ready matmuls.
                for args in state["pending"]:
                    rope_tail(*args)
                state["pending"] = []

            def chain_tail(e, ps, raw_dve):
                # Emit the psum->bf16 copy and the non-rot half of RoPE
                # (t1 = ps*cos) immediately at chain end, high priority:
                # together they release the chain's psum buf fast, which the
                # attn pipeline's STs recycle.  raw_dve: chains interleaved
                # into the attn phase use DVE for the copy so it doesn't
                # queue behind the attention exps on ACT.
                raw = workp.tile([P, BK], bf, tag="raw", name="raw")
                t1 = workp.tile([P, BK], fp32, tag="t1", name="t1")
                with tc.high_priority():
                    if raw_dve:
                        nc.vector.tensor_copy(raw[:], ps[:])
                    else:
                        nc.scalar.copy(raw[:], ps[:])
                    nc.vector.tensor_mul(t1[:], ps[:], cs_sb[:, 0, tsl])
                state["pending"].append((e, t1, raw))

            def e_chain(e, raw_dve=False, no_flush=False):
                ps = psmm.tile([P, BK], fp32, tag="mm", name="ps_qk")
                for kt in range(NKT):
                    nc.tensor.matmul(
                        ps[:],
                        wqk_sb[:, e, kt, :],
                        xt_sb[:, kt, :],
                        start=(kt == 0), stop=(kt == NKT - 1),
                    )
                if not no_flush:
                    flush_rope()
                chain_tail(e, ps, raw_dve)

            def e_chain_pair_paced(ea, eb):
                # block 0 only: the first chains race the xt DMA stream
                # (~357ns/chunk vs 216ns/MM), so run the two heads' chains
                # kt-interleaved to consume each chunk at DMA pace.
                psa = psmm.tile([P, BK], fp32, tag="mm", name="ps_qk")
                psb = psmm.tile([P, BK], fp32, tag="mm", name="ps_qk")
                for kt in range(NKT):
                    nc.tensor.matmul(psa[:], wqk_sb[:, ea, kt, :],
                                     xt_sb[:, kt, :],
                                     start=(kt == 0), stop=(kt == NKT - 1))
                    nc.tensor.matmul(psb[:], wqk_sb[:, eb, kt, :],
                                     xt_sb[:, kt, :],
                                     start=(kt == 0), stop=(kt == NKT - 1))
                chain_tail(ea, psa, False)
                chain_tail(eb, psb, False)

            def v_chain(tt, jv=None, on_dve=False):
                # v for block jv (this block, or j+1 when interleaved into
                # the attn phase as PE filler work; then the psum copy goes
                # on DVE since ACT is busy with attention exps)
                if jv is None:
                    jv = j
                src = xt_sb if jv == j else xt_next
                ps = psmm.tile([P, BK], fp32, tag="mm", name="ps_v")
                for kt in range(NKT):
                    nc.tensor.matmul(
                        ps[:],
                        src[:, kt, tt * P:(tt + 1) * P],
                        wv_sb[:, kt, :],
                        start=(kt == 0), stop=(kt == NKT - 1),
                    )
                flush_rope()
                # high priority: this copy releases the psmm buf the next
                # attn head's STs need; queued normally it drains ~2-3us
                # late behind the attn phase's bulk DVE/ACT work.
                with tc.high_priority():
                    if on_dve:
                        nc.vector.tensor_copy(v_sb[:, 4 * jv + tt, :], ps[:])
                    else:
                        nc.scalar.copy(v_sb[:, 4 * jv + tt, :], ps[:])

            def attn_head(h, psrot_ok=True):
                pso = psop.tile([HD, BK], fp32, tag="o", name="pso")
                pss = pssum.tile([P, BK], fp32, tag="sum", name="pss")
                # j>0: row sums via fp8 DoubleRow matmuls on s-tile pairs
                # (2x PE) from a Pool-made fp8 copy of pt.  j==0 rows have
                # few softmax terms, where fp8 sum error is largest -> bf16.
                use8 = j > 0
                npair = ni // 2
                pair_state = {}

                def st_stage(i):
                    r = i - 4 * j
                    t0 = P * max(r, 0)
                    # The head's FIRST ST borrows the rope bank: at segment
                    # boundaries all 3 psmm bufs are held by the preceding
                    # chains until their psum copies drain (~0.7us), which
                    # would stall the attn pipeline start.  Later STs use
                    # psmm (3 in flight); borrowing more would stall the
                    # interleaved ropes' rot matmuls on ST exps.  psrot_ok
                    # is False when ropes were just flushed at this boundary.
                    if i == 0 and psrot_ok:
                        pst = psrot.tile([P, BK], fp32, tag="rot", name="pst")
                    else:
                        pst = psmm.tile([P, BK], fp32, tag="mm", name="pst")
                    nc.tensor.matmul(
                        pst[:, t0:],
                        k_sb[:, h, i * P:(i + 1) * P],
                        q_sb[:, h, t0:],
                        start=True, stop=True,
                    )
                    pt = ptp.tile([P, BK], bf, tag="pt", name="pt")
                    nc.scalar.activation(pt[:, t0:], pst[:, t0:], Exp,
                                         scale=SCALE)
                    if r >= 0:
                        nc.vector.tensor_mul(
                            pt[:, t0:t0 + P], pt[:, t0:t0 + P], cst_sb[:, 1, :]
                        )
                    pt8 = None
                    if use8 and r < 0:
                        # fp8 copies for full (below-diagonal) tiles
                        slot = i % 2
                        if slot == 0:
                            pair_state["pt8"] = pt8p.tile(
                                [P, 2, BK], e4, tag="pt8", name="pt8")
                        p8 = pair_state["pt8"]
                        nc.vector.tensor_scalar_mul(p8[:, slot, :],
                                                    pt[:], E4SCALE)
                        if slot == 1:
                            pt8 = p8
                    elif use8:
                        # diag tiles (j>=1 rows have >=512 softmax terms, so
                        # fp8 sum error is fine): pair (r0,r1) DRs over
                        # [P:BK), (r2,r3) over [3P:BK); the uncovered strips
                        # keep small bf16 ones matmuls in pv_stage.  Copies
                        # read pt AFTER the tri-mask mul above.
                        base = P if r < 2 else 3 * P
                        slot = r % 2
                        if slot == 0:
                            pair_state["pt8d"] = pt8p.tile(
                                [P, 2, BK], e4, tag="pt8", name="pt8d")
                        p8d = pair_state["pt8d"]
                        nc.vector.tensor_scalar_mul(p8d[:, slot, base:],
                                                    pt[:, base:], E4SCALE)
                        if slot == 1:
                            pt8 = p8d
                    return (i, t0, pt, pt8)

                def pv_stage(i, t0, pt, pt8):
                    # pss is ONE accumulation group: the bank's zero-pending
                    # is armed exactly once (first matmul start=True); later
                    # matmuls overwrite-on-first-touch then accumulate, so
                    # DR pair chunks and bf16 diag matmuls interleave freely.
                    r = i - 4 * j
                    if not use8:
                        nc.tensor.matmul(
                            pss[:, t0:], cst_sb[:, 2, :], pt[:, t0:],
                            start=(i == 0), stop=(i == ni - 1),
                        )
                    elif r < 0:
                        if pt8 is not None:
                            pc = i // 2
                            for n0 in (0, 256):
                                nc.tensor.matmul(
                                    pss[:, n0:n0 + 256], ones8_sb[:],
                                    pt8[:, :, n0:n0 + 256],
                                    start=(pc == 0 and n0 == 0), stop=False,
                                    perf_mode=DR, skip_group_check=True,
                                )
                    elif r == 0:
                        # strip [0:P): only r0 contributes below the pair
                        nc.tensor.matmul(
                            pss[:, 0:P], cst_sb[:, 2, :], pt[:, 0:P],
                            start=False, stop=False, skip_group_check=True,
                        )
                    elif r == 1:
                        # DR pair (r0,r1) over [P:BK) in <=256-col chunks
                        for n0 in (P, P + 256):
                            n1 = min(n0 + 256, BK)
                            nc.tensor.matmul(
                                pss[:, n0:n1], ones8_sb[:],
                                pt8[:, :, n0:n1],
                                start=False, stop=False,
                                perf_mode=DR, skip_group_check=True,
                            )
                    elif r == 2:
                        # strip [2P:3P): only r2 contributes there
                        nc.tensor.matmul(
                            pss[:, 2 * P:3 * P], cst_sb[:, 2, :],
                            pt[:, 2 * P:3 * P],
                            start=False, stop=False, skip_group_check=True,
                        )
                    else:
                        # DR pair (r2,r3) over [3P:BK)
                        nc.tensor.matmul(
                            pss[:, 3 * P:], ones8_sb[:], pt8[:, :, 3 * P:],
                            start=False, stop=(i == ni - 1),
                            perf_mode=DR, skip_group_check=True,
                        )
                    nc.tensor.matmul(
                        pso[:, t0:], v_sb[:, i, h * HD:(h + 1) * HD],
                        pt[:, t0:],
                        start=(i == 0), stop=(i == ni - 1),
                    )

                fifo = deque()
                for i in range(ni):
                    fifo.append(st_stage(i))
                    if len(fifo) > 2:
                        pv_stage(*fifo.popleft())
                while fifo:
                    pv_stage(*fifo.popleft())

                recip = workp.tile([P, BK], fp32, tag="recip", name="recip")
                if h == HPG - 1:
                    # the last head's normalize gates the whole proj phase
                    with tc.high_priority():
                        nc.vector.reciprocal_approx_fast(recip[:], pss[:])
                        nc.vector.tensor_mul(ot_sb[:, h, :], pso[:], recip[:])
                else:
                    nc.vector.reciprocal_approx_fast(recip[:], pss[:])
                    nc.vector.tensor_mul(ot_sb[:, h, :], pso[:], recip[:])

            def prefetch_xt():
                nonlocal xt_next
                if j + 1 < NB:
                    xt_next = xtp.tile([P, NKT, BK], bf, tag="xt",
                                       name=f"xt_sb{j + 1}")
                    nc.sync.dma_start(xt_next[:], xt_d[j + 1, :, :, :])

            def proj_phase():
                for tt in range(BK // P):
                    for n in range(D // BK):
                        psy = psmm.tile([P, BK], fp32, tag="mm", name="psy")
                        for h in range(HPG):
                            nc.tensor.matmul(
                                psy[:],
                                ot_sb[:, h, tt * P:(tt + 1) * P],
                                wp_sb[:, h, n * BK:(n + 1) * BK],
                                start=(h == 0), stop=(h == HPG - 1),
                            )
                        y = ypool.tile([P, BK], bf, tag="y", name="y")
                        if (tt * (D // BK) + n) % 2 == 0:
                            nc.scalar.copy(y[:], psy[:])
                        else:
                            nc.vector.tensor_copy(y[:], psy[:])
                        nc.sync.dma_start(
                            out_d[(j * 4 + tt) * P:(j * 4 + tt + 1) * P,
                                  n * BK:(n + 1) * BK],
                            y[:],
                        )

            # Block body: the attention phase is ACT(exp)-throughput-bound
            # relative to PE, so PE-only filler work (next head-pair's qkv
            # chains, next block's v-chains) is interleaved between attention
            # heads -- ACT catches up during the fillers, and the rope for
            # each head's q/k lands >=1 full chain before its first use.
            # Flush choreography: ropes are flushed at the FIRST chain after
            # each attn head (never at the chain right before a head), so no
            # rot matmul churns the psrot bank at a head boundary where the
            # head's first ST borrows it.
            if j == 0:
                e_chain_pair_paced(0, 4)            # pends r0, r4
                e_chain(1)                          # flush r0, r4
                e_chain(5, no_flush=True)           # pends r1, r5
                v_chain(0)                          # flush r1, r5
                for tt in range(1, BK // P):
                    v_chain(tt)
            else:
                prefetch_xt()
                e_chain(0)
                e_chain(4)                          # flush r0
                e_chain(1)                          # flush r4
                e_chain(5, no_flush=True)           # pends r1, r5
            attn_head(0)
            e_chain(2, raw_dve=True)                # flush r1, r5
            e_chain(6, raw_dve=True, no_flush=True)  # pends r2, r6
            attn_head(1)
            e_chain(3, raw_dve=True)                # flush r2, r6
            last = j + 1 >= NB
            e_chain(7, raw_dve=True, no_flush=not last)
            if last:
                flush_rope()                        # r3, r7 at e7 end
            attn_head(2, psrot_ok=not last)
            if not last:
                v_chain(0, jv=j + 1, on_dve=True)   # flush r3, r7
                v_chain(1, jv=j + 1, on_dve=True)
            attn_head(3)
            if not last:
                v_chain(2, jv=j + 1, on_dve=True)
                v_chain(3, jv=j + 1, on_dve=True)
            proj_phase()

    nc.compile()
    return nc


def _get_nc():
    if "nc" not in _NC_CACHE:
        _NC_CACHE["nc"] = _build_nc()
    return _NC_CACHE["nc"]


def _host_prep(x, Wqkv, Wproj, mask):
    """Build the 8 per-core input maps (host-side layout transforms)."""
    x = np.asarray(x, dtype=np.float32)
    Wqkv = np.asarray(Wqkv, dtype=np.float32)
    Wproj = np.asarray(Wproj, dtype=np.float32)
    mask = np.asarray(mask, dtype=np.float32)

    # RoPE tables (transposed layout [hd, T]); matches the standard
    # rotate-half RoPE with base 10000.
    inv_freq = 1.0 / (10000.0 ** (np.arange(0, HD, 2, dtype=np.float32) / HD))
    freqs = np.arange(T, dtype=np.float32)[:, None] * inv_freq[None, :]
    emb = np.concatenate([freqs, freqs], axis=-1)        # [T, 128]
    cosT = np.ascontiguousarray(np.cos(emb).T).astype(bf16)
    sinT_f = np.ascontiguousarray(np.sin(emb).T)
    sinT_f[:HD // 2] *= -1.0
    sinT = sinT_f.astype(bf16)
    cs = np.ascontiguousarray(np.stack([cosT, sinT], axis=1))  # [HD, 2, T]

    # rotate-half permutation (signs folded): out[d] = -in[d+64] (d<64),
    # +in[d-64] (d>=64); lhsT layout [K=i, M=d].
    rot = np.zeros((HD, HD), dtype=np.float32)
    for d in range(HD // 2):
        rot[d + HD // 2, d] = 1.0
    for d in range(HD // 2, HD):
        rot[d - HD // 2, d] = 1.0
    rot = rot.astype(bf16)

    # [s', t''] triangle for the diagonal 128x128 tile, from the real mask
    tri = (mask[:P, :P].T == 0.0).astype(bf16)
    ones = np.ones((P, P), dtype=bf16)
    cst = np.ascontiguousarray(np.stack([rot, tri, ones], axis=1))  # [P,3,P]
    ones8 = np.full((P, 2, P), ONES8VAL, dtype=f8e4)

    in_maps = []
    for b in range(B):
        xT = np.ascontiguousarray(x[b].T).astype(bf16)      # [D, T]
        # [NB, P, NKT, BK]: per (j, partition) rows are 16KB contiguous
        xt_pre = np.ascontiguousarray(
            xT.reshape(NKT, P, NB, BK).transpose(2, 1, 0, 3))
        for g in range(NG):
            heads = list(range(HPG * g, HPG * (g + 1)))
            wq = [Wqkv[:, h * HD:(h + 1) * HD] for h in heads]
            wk = [Wqkv[:, D + h * HD:D + (h + 1) * HD] for h in heads]
            wvl = [Wqkv[:, 2 * D + h * HD:2 * D + (h + 1) * HD] for h in heads]
            # [8, P, NKT, HD]: per-(e,partition) rows 4KB contiguous
            wqk = np.ascontiguousarray(
                np.stack(wq + wk, axis=0).astype(bf16)
                .reshape(2 * HPG, NKT, P, HD).transpose(0, 2, 1, 3))
            wv = np.ascontiguousarray(
                np.concatenate(wvl, axis=1).astype(bf16)
                .reshape(NKT, P, HPG * HD).transpose(1, 0, 2))
            wp = np.ascontiguousarray(
                Wproj[HPG * HD * g:HPG * HD * (g + 1), :].astype(bf16)
                .reshape(HPG, P, D).transpose(1, 0, 2))
            in_maps.append({
                "xt": xt_pre, "wqk": wqk, "wv": wv, "wp": wp,
                "cs": cs, "cst": cst, "ones8": ones8,
            })
    return in_maps


def run(x, Wqkv, Wproj, bproj, mask, trace=False):
    """Run the SPMD kernel; returns (output, BassKernelResults)."""
    from concourse.bass_utils import run_bass_kernel_spmd

    nc = _get_nc()
    in_maps = _host_prep(x, Wqkv, Wproj, mask)
    res = run_bass_kernel_spmd(nc, in_maps, core_ids=list(range(B * NG)),
                               trace=trace)

    bproj = np.asarray(bproj, dtype=np.float32)
    out = np.zeros((B, T, D), dtype=np.float32)
    for b in range(B):
        acc = np.zeros((T, D), dtype=np.float32)
        for g in range(NG):
            acc += np.asarray(res.results[b * NG + g]["out"], dtype=np.float32)
        out[b] = acc + bproj[None, :]
    return out, res


def kernel(x, Wqkv, Wproj, bproj, mask):
    # The very first execution in a fresh process occasionally returns
    # corrupted output (cold-start device/upload race in the runtime, seen
    # ~1-in-5 across sessions; identical reruns are clean).  Validate and
    # retry: output elements are bounded (~|out| < 100 for this problem), so
    # non-finite values or absurd magnitudes mean a corrupted run.
    out = None
    err = None
    for _attempt in range(3):
        try:
            out, _ = run(x, Wqkv, Wproj, bproj, mask, trace=False)
        except Exception as e:  # transient device error: retry
            err = e
            continue
        if np.isfinite(out).all() and np.abs(out).max() < 1e4:
            break
    if out is None:
        raise err
    return out

